# revision 7
# baseline (speedup 1.0000x reference)
"""Local (sliding-window) self-attention Trainium2 kernel.

Problem: B=1, L=2048, EMBED=768, HEADS=12, HEAD_DIM=64, WINDOW=65 (pad 32).
Sharding: sequence dim L split across 8 NeuronCores (256 tokens each) with a
32-token halo on each side (zero-padded at sequence ends, matching the
reference's zero-pad semantics: padded positions have k=v=0 and participate
in the softmax with score 0).

Per-core pipeline (no collectives):
  1. qkT[h] = (x @ Wqk_h)^T  -> [64q+64k, 320]   (lhsT = w_qkv slice, rhs = x^T)
  2. v     = x @ Wv          -> [320, 768]        (lhsT = x^T chunk, rhs = w_qkv v-cols)
  3. per (head, 128-query block):
       sT      = k_win @ q^T            [192, 128] (2 matmuls, transposed scores)
       e       = exp(sT / 8)            (ACT, PSUM->SBUF)
       e      *= band_mask              (DVE; exact zeros outside the window)
       den     = ones^T @ e             [1, 128]   (PE reduction over partitions)
       rden    = 1/den                  (DVE reciprocal)
       rb      = ones64 (x) rden        [64, 128]  (K=1 matmul broadcast)
       oT      = v_win^T @ e            [64, 128]  (2 accumulating matmuls)
       oT     *= rb                     (DVE, written into head-pair tile)
  4. out = attn^T.T @ w_out + b_out     (bias via K=1 ones (x) b_out matmul)
"""

import os
import sys

import numpy as np

for _p in ("/opt/trn_rl_repo",):
    if os.path.isdir(_p) and _p not in sys.path:
        sys.path.insert(0, _p)

import concourse.bass as bass  # noqa: E402
import concourse.bacc as bacc  # noqa: E402
import concourse.tile as tile  # noqa: E402
from concourse import mybir  # noqa: E402
from concourse import bass_utils  # noqa: E402

HEADS = 12
DH = 64
WINDOW = 65
PAD = WINDOW // 2  # 32
E = 768
L = 2048
N_CORES = 8
LLOC = L // N_CORES           # 256 queries per core
LEXT = LLOC + 2 * PAD         # 320 token rows per core (with halo)
KC = E // 128                 # 6 contraction chunks
NBLK = LLOC // 128            # 2 query blocks of 128
F32 = mybir.dt.float32

_CACHE = {}


def _build_module(reps=1):
    nc = bacc.Bacc(
        "TRN2",
        target_bir_lowering=False,
        debug=False,
        num_devices=N_CORES,
    )
    xT = nc.dram_tensor("xT", [E, LEXT], F32, kind="ExternalInput").ap()
    wqkv = nc.dram_tensor("w_qkv", [E, 3 * HEADS * DH], F32, kind="ExternalInput").ap()
    wout = nc.dram_tensor("w_out", [HEADS * DH, E], F32, kind="ExternalInput").ap()
    bout = nc.dram_tensor("b_out", [E], F32, kind="ExternalInput").ap()
    mhi_d = nc.dram_tensor("mask_hi", [128, 128], F32, kind="ExternalInput").ap()
    mlo_d = nc.dram_tensor("mask_lo", [64, 128], F32, kind="ExternalInput").ap()
    out_d = nc.dram_tensor("out", [LLOC, E], F32, kind="ExternalOutput").ap()

    EXPF = mybir.ActivationFunctionType.Exp

    with tile.TileContext(nc) as tc:
        with (
            tc.tile_pool(name="wq", bufs=KC) as wq_pool,
            tc.tile_pool(name="wo", bufs=KC) as wo_pool,
            tc.tile_pool(name="xt", bufs=KC) as xt_pool,
            tc.tile_pool(name="const", bufs=1) as cpool,
            tc.tile_pool(name="qk", bufs=2 * HEADS) as qk_pool,
            tc.tile_pool(name="vsb", bufs=3) as v_pool,
            tc.tile_pool(name="ex", bufs=3) as ex_pool,
            tc.tile_pool(name="opair", bufs=KC) as op_pool,
            tc.tile_pool(name="rb", bufs=2) as rb_pool,
            tc.tile_pool(name="rden", bufs=2) as rden_pool,
            tc.tile_pool(name="fin", bufs=2) as fin_pool,
            tc.tile_pool(name="psA", bufs=2, space="PSUM") as psA,
            tc.tile_pool(name="psS", bufs=2, space="PSUM") as psS,
            tc.tile_pool(name="psD", bufs=1, space="PSUM") as psD,
            tc.tile_pool(name="psR", bufs=1, space="PSUM") as psR,
            tc.tile_pool(name="psO", bufs=2, space="PSUM") as psO,
        ):
            # ---- constants ----
            mhi = cpool.tile([128, 128], F32, tag="mhi")
            nc.sync.dma_start(mhi[:], mhi_d[:])
            mlo = cpool.tile([64, 128], F32, tag="mlo")
            nc.sync.dma_start(mlo[:], mlo_d[:])
            ones_col = cpool.tile([128, 1], F32, tag="onesc")
            nc.vector.memset(ones_col[:], 1.0)
            ones_row = cpool.tile([1, 128], F32, tag="onesr")
            nc.vector.memset(ones_row[:], 1.0)
            b_sb = cpool.tile([1, E], F32, tag="bout")
            nc.sync.dma_start(b_sb[0:1, :], bout[None, :])

            for rep in range(reps):
                _emit_body(nc, tc, locals())

    nc.compile()
    return nc


def _emit_body(nc, tc, env):
    (xT, wqkv, wout, bout, mhi_d, mlo_d, out_d) = (
        env["xT"], env["wqkv"], env["wout"], env["bout"],
        env["mhi_d"], env["mlo_d"], env["out_d"],
    )
    (wq_pool, wo_pool, xt_pool, qk_pool, v_pool, ex_pool, op_pool,
     rb_pool, rden_pool, fin_pool, psA, psS, psD, psR, psO) = (
        env["wq_pool"], env["wo_pool"], env["xt_pool"], env["qk_pool"],
        env["v_pool"], env["ex_pool"], env["op_pool"], env["rb_pool"],
        env["rden_pool"], env["fin_pool"], env["psA"], env["psS"],
        env["psD"], env["psR"], env["psO"],
    )
    mhi, mlo, ones_col, ones_row, b_sb = (
        env["mhi"], env["mlo"], env["ones_col"], env["ones_row"], env["b_sb"]
    )
    rep = env["rep"]
    EXPF = mybir.ActivationFunctionType.Exp
    if True:
        if True:
            # ---- input DMAs ----
            xt_sb = []
            for c in range(KC):
                t = xt_pool.tile([128, LEXT], F32, tag="xt")
                nc.sync.dma_start(t[:], xT[c * 128:(c + 1) * 128, :])
                xt_sb.append(t)
            wq_sb = []
            for c in range(KC):
                t = wq_pool.tile([128, 3 * HEADS * DH], F32, tag="wq")
                nc.sync.dma_start(t[:], wqkv[c * 128:(c + 1) * 128, :])
                wq_sb.append(t)
            wo_sb = []
            for c in range(KC):
                t = wo_pool.tile([128, E], F32, tag="wo")
                nc.sync.dma_start(t[:], wout[c * 128:(c + 1) * 128, :])
                wo_sb.append(t)

            # ---- 1. q/k projection (transposed layout) ----
            qTs, kTs = [], []
            for h in range(HEADS):
                ps = psA.tile([128, LEXT], F32, tag="proj")
                for c in range(KC):
                    nc.tensor.matmul(
                        ps[:],
                        wq_sb[c][:, h * 3 * DH:h * 3 * DH + 2 * DH],
                        xt_sb[c][:],
                        start=(c == 0),
                        stop=(c == KC - 1),
                    )
                qT = qk_pool.tile([64, LEXT], F32, tag="qk")
                nc.any.tensor_copy(qT[:], ps[0:DH, :])
                kT = qk_pool.tile([64, LEXT], F32, tag="qk")
                nc.any.tensor_copy(kT[:], ps[DH:2 * DH, :])
                qTs.append(qT)
                kTs.append(kT)

            # ---- 2. v projection (natural layout), all heads batched ----
            v_sb = []
            for t_i in range(3):
                P = 128 if t_i < 2 else 64
                vt = v_pool.tile([P, HEADS * DH], F32, tag="vsb")
                for g in range(2):  # two groups of 6 heads, N=384 each
                    vps = psA.tile([128, 384], F32, tag="proj")
                    for c in range(KC):
                        w3 = wq_sb[c][:].rearrange("p (h f) -> p h f", h=HEADS)
                        nc.tensor.matmul(
                            vps[0:P, :],
                            xt_sb[c][:, t_i * 128:t_i * 128 + P],
                            w3[:, 6 * g:6 * (g + 1), 2 * DH:3 * DH],
                            start=(c == 0),
                            stop=(c == KC - 1),
                        )
                    nc.any.tensor_copy(vt[:, g * 384:(g + 1) * 384], vps[0:P, :])
                v_sb.append(vt)

            # ---- 3. banded attention ----
            op_tiles = [
                op_pool.tile([128, LLOC], F32, tag="opair", name=f"opair{rep}_{i}")
                for i in range(KC)
            ]
            for h in range(HEADS):
                for b in range(NBLK):
                    sps = psS.tile([128, 256], F32, tag="s")
                    q_sl = qTs[h][:, PAD + b * 128:PAD + b * 128 + 128]
                    nc.tensor.matmul(
                        sps[:, 0:128],
                        kTs[h][:, b * 128:b * 128 + 128],
                        q_sl,
                        start=True,
                        stop=True,
                    )
                    nc.tensor.matmul(
                        sps[0:64, 128:256],
                        kTs[h][:, b * 128 + 128:b * 128 + 192],
                        q_sl,
                        start=True,
                        stop=True,
                    )
                    ex = ex_pool.tile([128, 256], F32, tag="ex")
                    nc.scalar.activation(ex[:, 0:128], sps[:, 0:128], EXPF, scale=0.125)
                    nc.scalar.activation(
                        ex[0:64, 128:256], sps[0:64, 128:256], EXPF, scale=0.125
                    )
                    nc.vector.tensor_mul(ex[:, 0:128], ex[:, 0:128], mhi[:])
                    nc.vector.tensor_mul(ex[0:64, 128:256], ex[0:64, 128:256], mlo[:])

                    dps = psD.tile([1, 128], F32, tag="den")
                    nc.tensor.matmul(
                        dps[:], ones_col[:, 0:1], ex[:, 0:128], start=True, stop=False
                    )
                    nc.tensor.matmul(
                        dps[:], ones_col[0:64, 0:1], ex[0:64, 128:256],
                        start=False, stop=True,
                    )
                    rden = rden_pool.tile([1, 128], F32, tag="rden")
                    nc.vector.reciprocal(rden[:], dps[:])
                    rbps = psR.tile([64, 128], F32, tag="rb")
                    nc.tensor.matmul(
                        rbps[:], ones_row[0:1, 0:64], rden[:], start=True, stop=True
                    )
                    rb = rb_pool.tile([64, 128], F32, tag="rb")
                    nc.any.tensor_copy(rb[:], rbps[:])

                    ops = psO.tile([64, 128], F32, tag="o")
                    nc.tensor.matmul(
                        ops[:],
                        v_sb[b][:, h * DH:(h + 1) * DH],
                        ex[:, 0:128],
                        start=True,
                        stop=False,
                    )
                    nc.tensor.matmul(
                        ops[:],
                        v_sb[b + 1][0:64, h * DH:(h + 1) * DH],
                        ex[0:64, 128:256],
                        start=False,
                        stop=True,
                    )
                    opt = op_tiles[h // 2]
                    nc.vector.tensor_mul(
                        opt[(h % 2) * 64:(h % 2) * 64 + 64, b * 128:(b + 1) * 128],
                        ops[:],
                        rb[:],
                    )

            # ---- 4. output projection ----
            for t_i in range(2):
                fsb = fin_pool.tile([128, E], F32, tag="fin")
                for n0, nw in ((0, 512), (512, 256)):
                    fps = psA.tile([128, 512], F32, tag="proj")
                    for c in range(KC):
                        nc.tensor.matmul(
                            fps[:, 0:nw],
                            op_tiles[c][:, t_i * 128:(t_i + 1) * 128],
                            wo_sb[c][:, n0:n0 + nw],
                            start=(c == 0),
                            stop=False,
                        )
                    nc.tensor.matmul(
                        fps[:, 0:nw],
                        ones_row[0:1, 0:128],
                        b_sb[0:1, n0:n0 + nw],
                        start=False,
                        stop=True,
                    )
                    nc.any.tensor_copy(fsb[:, n0:n0 + nw], fps[:, 0:nw])
                nc.sync.dma_start(out_d[t_i * 128:(t_i + 1) * 128, :], fsb[:])


def _get_module(reps=1):
    if reps not in _CACHE:
        _CACHE[reps] = _build_module(reps)
    return _CACHE[reps]


def _make_masks():
    j = np.arange(192)[:, None]
    i = np.arange(128)[None, :]
    m = ((j >= i) & (j <= i + 2 * PAD)).astype(np.float32)
    return np.ascontiguousarray(m[:128]), np.ascontiguousarray(m[128:])


def _prepare_in_maps(x, w_qkv, w_out, b_out):
    x2 = np.asarray(x, dtype=np.float32).reshape(L, E)
    xp = np.zeros((L + 2 * PAD, E), dtype=np.float32)
    xp[PAD:PAD + L] = x2
    mhi, mlo = _make_masks()
    w_qkv = np.ascontiguousarray(np.asarray(w_qkv, dtype=np.float32))
    w_out = np.ascontiguousarray(np.asarray(w_out, dtype=np.float32))
    b_out = np.ascontiguousarray(np.asarray(b_out, dtype=np.float32))
    in_maps = []
    for i in range(N_CORES):
        xTi = np.ascontiguousarray(xp[i * LLOC:i * LLOC + LEXT].T)
        in_maps.append(
            {
                "xT": xTi,
                "w_qkv": w_qkv,
                "w_out": w_out,
                "b_out": b_out,
                "mask_hi": mhi,
                "mask_lo": mlo,
            }
        )
    return in_maps


def _run(in_maps, trace=False, trace_cores=None, reps=1):
    nc = _get_module(reps)
    return bass_utils.run_bass_kernel_spmd(
        nc,
        in_maps,
        core_ids=list(range(N_CORES)),
        trace=trace,
        trace_cores=trace_cores,
    )


def kernel(x, w_qkv, w_out, b_out):
    res = _run(_prepare_in_maps(x, w_qkv, w_out, b_out))
    y = np.concatenate([res.results[i]["out"] for i in range(N_CORES)], axis=0)
    return y.reshape(1, L, E).astype(np.float32)


def kernel_profiled(x, w_qkv, w_out, b_out, trace_cores=None):
    res = _run(
        _prepare_in_maps(x, w_qkv, w_out, b_out),
        trace=True,
        trace_cores=trace_cores,
    )
    y = np.concatenate([res.results[i]["out"] for i in range(N_CORES)], axis=0)
    return y.reshape(1, L, E).astype(np.float32), res


# revision 11
# speedup vs baseline: 6.6721x; 6.6721x over previous
"""Local (sliding-window) self-attention Trainium2 kernel.

Problem: B=1, L=2048, EMBED=768, HEADS=12, HEAD_DIM=64, WINDOW=65 (pad 32).
Sharding: sequence dim L split across 8 NeuronCores (256 tokens each) with a
32-token halo on each side (zero-padded at sequence ends, matching the
reference's zero-pad semantics: padded positions have k=v=0 and participate
in the softmax with score 0).

Per-core pipeline (no collectives; all matmul operands are float32r views —
TF32-like PE mode, 4x throughput at free-dim >= 256):
  1. qkT[h] = (x @ Wqk_h)^T  -> [64q+64k, 320]   (lhsT = w_qkv slice, rhs = x^T)
  2. v     = x @ Wv          -> [320, 768]        (lhsT = x^T chunk, rhs = w_qkv v-cols)
  3. per head, one 256-query block whose 320-key window = 3 partition chunks
     (128/128/64 rows, aligned with the v tiles):
       sT_c    = k_chunk @ q^T          [P, 256]   (3 matmuls)
       e_c     = exp(sT_c / 8)          (ACT, PSUM->SBUF, f32r out)
       e_c    *= band_mask_c            (DVE; exact zeros outside the window)
       den     = ones^T @ e             [1, 256]   (3 accumulating matmuls)
       rden    = 1/den                  (DVE reciprocal)
       rb      = ones64 (x) rden        [64, 256]  (K=1 matmul broadcast)
       oT      = v_win^T @ e            [64, 256]  (3 accumulating matmuls)
       oT     *= rb                     (DVE, written into head-pair tile)
  4. out = attn^T.T @ w_out + b_out     (bias via K=1 ones (x) b_out matmul)
"""

import os
import sys

import numpy as np

for _p in ("/opt/trn_rl_repo",):
    if os.path.isdir(_p) and _p not in sys.path:
        sys.path.insert(0, _p)

import concourse.bass as bass  # noqa: E402
import concourse.bacc as bacc  # noqa: E402
import concourse.tile as tile  # noqa: E402
from concourse import mybir  # noqa: E402
from concourse import bass_utils  # noqa: E402

HEADS = 12
DH = 64
WINDOW = 65
PAD = WINDOW // 2  # 32
E = 768
L = 2048
N_CORES = 8
LLOC = L // N_CORES           # 256 queries per core
LEXT = LLOC + 2 * PAD         # 320 token rows per core (with halo)
KC = E // 128                 # 6 contraction chunks
NBLK = LLOC // 128            # (kept for test.py's emulation)
F32 = mybir.dt.float32
F32R = mybir.dt.float32r

_CACHE = {}

# window chunks: (row offset, rows) — aligned with the three v tiles
WCHUNKS = ((0, 128), (128, 128), (256, 64))


def _build_module(reps=1, loop_iters=None):
    nc = bacc.Bacc(
        "TRN2",
        target_bir_lowering=False,
        debug=False,
        num_devices=N_CORES,
    )
    xT = nc.dram_tensor("xT", [E, LEXT], F32, kind="ExternalInput").ap()
    wqkv = nc.dram_tensor("w_qkv", [E, 3 * HEADS * DH], F32, kind="ExternalInput").ap()
    wout = nc.dram_tensor("w_out", [HEADS * DH, E], F32, kind="ExternalInput").ap()
    bout = nc.dram_tensor("b_out", [E], F32, kind="ExternalInput").ap()
    mask_d = [
        nc.dram_tensor(f"mask{c}", [p, LLOC], F32, kind="ExternalInput").ap()
        for c, (_, p) in enumerate(WCHUNKS)
    ]
    onesc_d = nc.dram_tensor("ones_col", [128, 1], F32, kind="ExternalInput").ap()
    onesr_d = nc.dram_tensor("ones_row", [1, 128], F32, kind="ExternalInput").ap()
    out_d = nc.dram_tensor("out", [LLOC, E], F32, kind="ExternalOutput").ap()

    with tile.TileContext(nc) as tc:
        with (
            tc.tile_pool(name="wq", bufs=KC) as wq_pool,
            tc.tile_pool(name="wo", bufs=KC) as wo_pool,
            tc.tile_pool(name="xt", bufs=KC) as xt_pool,
            tc.tile_pool(name="const", bufs=1) as cpool,
            tc.tile_pool(name="qk", bufs=2 * HEADS) as qk_pool,
            tc.tile_pool(name="vsb", bufs=3) as v_pool,
            tc.tile_pool(name="ex", bufs=6) as ex_pool,
            tc.tile_pool(name="opair", bufs=KC) as op_pool,
            tc.tile_pool(name="rb", bufs=2) as rb_pool,
            tc.tile_pool(name="rden", bufs=2) as rden_pool,
            tc.tile_pool(name="fin", bufs=2) as fin_pool,
            tc.tile_pool(name="psA", bufs=2, space="PSUM") as psA,
            tc.tile_pool(name="psS", bufs=3, space="PSUM") as psS,
            tc.tile_pool(name="psD", bufs=1, space="PSUM") as psD,
            tc.tile_pool(name="psO", bufs=2, space="PSUM") as psO,
        ):
            # ---- constants ----
            masks = []
            for c, (_, p) in enumerate(WCHUNKS):
                m = cpool.tile([p, LLOC], F32R, tag=f"mask{c}", name=f"mask{c}")
                nc.sync.dma_start(m[:], mask_d[c].bitcast(F32R))
                masks.append(m)
            ones_col = cpool.tile([128, 1], F32R, tag="onesc")
            nc.sync.dma_start(ones_col[:], onesc_d.bitcast(F32R))
            ones_row = cpool.tile([1, 128], F32R, tag="onesr")
            nc.sync.dma_start(ones_row[:], onesr_d.bitcast(F32R))
            b_sb = cpool.tile([1, E], F32R, tag="bout")
            nc.sync.dma_start(b_sb[0:1, :], bout[None, :].bitcast(F32R))

            if loop_iters is None:
                for rep in range(reps):
                    _emit_body(nc, tc, locals())
            else:
                rep = 0
                with tc.For_i(
                    0, loop_iters, 1,
                    hint_engines=(mybir.EngineType.PE,
                                  mybir.EngineType.DVE,
                                  mybir.EngineType.Activation),
                ) as _i:
                    _emit_body(nc, tc, locals())

    nc.compile()
    return nc


def _emit_body(nc, tc, env):
    (xT, wqkv, wout, out_d) = (env["xT"], env["wqkv"], env["wout"], env["out_d"])
    (wq_pool, wo_pool, xt_pool, qk_pool, v_pool, ex_pool, op_pool,
     rb_pool, rden_pool, fin_pool, psA, psS, psD, psO) = (
        env["wq_pool"], env["wo_pool"], env["xt_pool"], env["qk_pool"],
        env["v_pool"], env["ex_pool"], env["op_pool"], env["rb_pool"],
        env["rden_pool"], env["fin_pool"], env["psA"], env["psS"],
        env["psD"], env["psO"],
    )
    masks, ones_col, ones_row, b_sb = (
        env["masks"], env["ones_col"], env["ones_row"], env["b_sb"]
    )
    rep = env["rep"]
    EXPF = mybir.ActivationFunctionType.Exp

    # ---- input DMAs ----
    xt_sb = []
    for c in range(KC):
        t = xt_pool.tile([128, LEXT], F32R, tag="xt", name=f"xt{rep}_{c}")
        nc.sync.dma_start(t[:], xT[c * 128:(c + 1) * 128, :].bitcast(F32R))
        xt_sb.append(t)
    wq_sb = []
    for c in range(KC):
        t = wq_pool.tile([128, 3 * HEADS * DH], F32R, tag="wq", name=f"wq{rep}_{c}")
        nc.sync.dma_start(t[:], wqkv[c * 128:(c + 1) * 128, :].bitcast(F32R))
        wq_sb.append(t)
    wo_sb = []
    for c in range(KC):
        t = wo_pool.tile([128, E], F32R, tag="wo", name=f"wo{rep}_{c}")
        nc.sync.dma_start(t[:], wout[c * 128:(c + 1) * 128, :].bitcast(F32R))
        wo_sb.append(t)

    # ---- 1. q/k projection (transposed layout) ----
    qTs, kTs = [], []
    for h in range(HEADS):
        ps = psA.tile([128, LEXT], F32, tag="proj", name=f"psqk{rep}_{h}")
        for c in range(KC):
            nc.tensor.matmul(
                ps[:],
                wq_sb[c][:, h * 3 * DH:h * 3 * DH + 2 * DH],
                xt_sb[c][:],
                start=(c == 0),
                stop=(c == KC - 1),
            )
        qT = qk_pool.tile([64, LLOC], F32R, tag="qk", name=f"qT{rep}_{h}")
        nc.any.tensor_copy(qT[:], ps[0:DH, PAD:PAD + LLOC])
        kT = qk_pool.tile([64, LEXT], F32R, tag="qk", name=f"kT{rep}_{h}")
        nc.any.tensor_copy(kT[:], ps[DH:2 * DH, :])
        qTs.append(qT)
        kTs.append(kT)

    # ---- 2. v projection (natural layout), all heads batched ----
    v_sb = []
    for t_i in range(3):
        P = 128 if t_i < 2 else 64
        vt = v_pool.tile([P, HEADS * DH], F32R, tag="vsb", name=f"v{rep}_{t_i}")
        for g in range(2):  # two groups of 6 heads, N=384 each
            vps = psA.tile([128, 384], F32, tag="proj", name=f"psv{rep}_{t_i}_{g}")
            for c in range(KC):
                w3 = wq_sb[c][:].rearrange("p (h f) -> p h f", h=HEADS)
                nc.tensor.matmul(
                    vps[0:P, :],
                    xt_sb[c][:, t_i * 128:t_i * 128 + P],
                    w3[:, 6 * g:6 * (g + 1), 2 * DH:3 * DH],
                    start=(c == 0),
                    stop=(c == KC - 1),
                )
            nc.any.tensor_copy(vt[:, g * 384:(g + 1) * 384], vps[0:P, :])
        v_sb.append(vt)

    # ---- 3. banded attention: one 256-query block per head ----
    op_tiles = [
        op_pool.tile([128, LLOC], F32R, tag="opair", name=f"opair{rep}_{i}")
        for i in range(KC)
    ]
    for h in range(HEADS):
        exs = []
        for c, (off, p) in enumerate(WCHUNKS):
            sps = psS.tile([128, LLOC], F32, tag="s", name=f"s{rep}_{h}_{c}")
            nc.tensor.matmul(
                sps[0:p, :],
                kTs[h][:, off:off + p],
                qTs[h][:],
                start=True,
                stop=True,
            )
            ex = ex_pool.tile([p, LLOC], F32R, tag="ex", name=f"ex{rep}_{h}_{c}")
            nc.scalar.activation(ex[:], sps[0:p, :], EXPF, scale=0.125)
            nc.vector.tensor_mul(ex[:], ex[:], masks[c][:])
            exs.append(ex)

        dps = psD.tile([1, LLOC], F32, tag="den", name=f"den{rep}_{h}")
        for c, (off, p) in enumerate(WCHUNKS):
            nc.tensor.matmul(
                dps[:],
                ones_col[0:p, 0:1],
                exs[c][:],
                start=(c == 0),
                stop=(c == 2),
            )
        rden = rden_pool.tile([1, LLOC], F32R, tag="rden", name=f"rden{rep}_{h}")
        with nc.allow_low_precision(reason="f32r view feeding PE broadcast"):
            nc.vector.reciprocal(rden[:], dps[:])
        rbps = psO.tile([64, LLOC], F32, tag="o", name=f"rbps{rep}_{h}")
        nc.tensor.matmul(
            rbps[:], ones_row[0:1, 0:64], rden[:], start=True, stop=True
        )
        rb = rb_pool.tile([64, LLOC], F32R, tag="rb", name=f"rb{rep}_{h}")
        nc.any.tensor_copy(rb[:], rbps[:])

        ops = psO.tile([64, LLOC], F32, tag="o", name=f"o{rep}_{h}")
        for c, (off, p) in enumerate(WCHUNKS):
            nc.tensor.matmul(
                ops[:],
                v_sb[c][:, h * DH:(h + 1) * DH],
                exs[c][:],
                start=(c == 0),
                stop=(c == 2),
            )
        opt = op_tiles[h // 2]
        nc.vector.tensor_mul(
            opt[(h % 2) * 64:(h % 2) * 64 + 64, :],
            ops[:],
            rb[:],
        )

    # ---- 4. output projection ----
    for t_i in range(2):
        fsb = fin_pool.tile([128, E], F32, tag="fin", name=f"fin{rep}_{t_i}")
        for n0, nw in ((0, 512), (512, 256)):
            fps = psA.tile([128, 512], F32, tag="proj", name=f"psf{rep}_{t_i}_{n0}")
            for c in range(KC):
                nc.tensor.matmul(
                    fps[:, 0:nw],
                    op_tiles[c][:, t_i * 128:(t_i + 1) * 128],
                    wo_sb[c][:, n0:n0 + nw],
                    start=(c == 0),
                    stop=False,
                )
            nc.tensor.matmul(
                fps[:, 0:nw],
                ones_row[0:1, 0:128],
                b_sb[0:1, n0:n0 + nw],
                start=False,
                stop=True,
            )
            nc.any.tensor_copy(fsb[:, n0:n0 + nw], fps[:, 0:nw])
        nc.sync.dma_start(out_d[t_i * 128:(t_i + 1) * 128, :], fsb[:])


def _get_module(reps=1, loop_iters=None):
    key = (reps, loop_iters)
    if key not in _CACHE:
        _CACHE[key] = _build_module(reps, loop_iters)
    return _CACHE[key]


def _make_masks():
    j = np.arange(LEXT)[:, None]
    i = np.arange(LLOC)[None, :]
    m = ((j >= i) & (j <= i + 2 * PAD)).astype(np.float32)
    return [np.ascontiguousarray(m[off:off + p]) for off, p in WCHUNKS]


def _prepare_in_maps(x, w_qkv, w_out, b_out):
    x2 = np.asarray(x, dtype=np.float32).reshape(L, E)
    xp = np.zeros((L + 2 * PAD, E), dtype=np.float32)
    xp[PAD:PAD + L] = x2
    masks = _make_masks()
    w_qkv = np.ascontiguousarray(np.asarray(w_qkv, dtype=np.float32))
    w_out = np.ascontiguousarray(np.asarray(w_out, dtype=np.float32))
    b_out = np.ascontiguousarray(np.asarray(b_out, dtype=np.float32))
    in_maps = []
    for i in range(N_CORES):
        xTi = np.ascontiguousarray(xp[i * LLOC:i * LLOC + LEXT].T)
        m = {
            "xT": xTi,
            "w_qkv": w_qkv,
            "w_out": w_out,
            "b_out": b_out,
        }
        for c in range(3):
            m[f"mask{c}"] = masks[c]
        m["ones_col"] = np.ones((128, 1), np.float32)
        m["ones_row"] = np.ones((1, 128), np.float32)
        in_maps.append(m)
    return in_maps


def _run(in_maps, trace=False, trace_cores=None, reps=1, loop_iters=None):
    nc = _get_module(reps, loop_iters)
    return bass_utils.run_bass_kernel_spmd(
        nc,
        in_maps,
        core_ids=list(range(N_CORES)),
        trace=trace,
        trace_cores=trace_cores,
    )


def kernel(x, w_qkv, w_out, b_out):
    res = _run(_prepare_in_maps(x, w_qkv, w_out, b_out))
    y = np.concatenate([res.results[i]["out"] for i in range(N_CORES)], axis=0)
    return y.reshape(1, L, E).astype(np.float32)


def kernel_profiled(x, w_qkv, w_out, b_out, trace_cores=None):
    res = _run(
        _prepare_in_maps(x, w_qkv, w_out, b_out),
        trace=True,
        trace_cores=trace_cores,
    )
    y = np.concatenate([res.results[i]["out"] for i in range(N_CORES)], axis=0)
    return y.reshape(1, L, E).astype(np.float32), res


# revision 12
# speedup vs baseline: 8.3115x; 1.2457x over previous
"""Local (sliding-window) self-attention Trainium2 kernel.

Problem: B=1, L=2048, EMBED=768, HEADS=12, HEAD_DIM=64, WINDOW=65 (pad 32).
Sharding: sequence dim L split across 8 NeuronCores (256 tokens each) with a
32-token halo on each side (zero-padded at sequence ends, matching the
reference's zero-pad semantics: padded positions have k=v=0 and participate
in the softmax with score 0).

Per-core pipeline (no collectives; all matmul operands are float32r views —
TF32-like PE mode, 4x throughput at free-dim >= 256):
  1. qkT[h] = (x @ Wqk_h)^T  -> [64q+64k, 320]   (lhsT = w_qkv slice, rhs = x^T)
  2. v     = x @ Wv          -> [320, 768]        (lhsT = x^T chunk, rhs = w_qkv v-cols)
  3. per head, one 256-query block whose 320-key window = 3 partition chunks
     (128/128/64 rows, aligned with the v tiles):
       sT_c    = k_chunk @ q^T          [P, 256]   (3 matmuls)
       e_c     = exp(sT_c / 8)          (ACT, PSUM->SBUF, f32r out)
       e_c    *= band_mask_c            (DVE; exact zeros outside the window)
       den     = ones^T @ e             [1, 256]   (3 accumulating matmuls)
       rden    = 1/den                  (DVE reciprocal)
       rb      = ones64 (x) rden        [64, 256]  (K=1 matmul broadcast)
       oT      = v_win^T @ e            [64, 256]  (3 accumulating matmuls)
       oT     *= rb                     (DVE, written into head-pair tile)
  4. out = attn^T.T @ w_out + b_out     (bias via K=1 ones (x) b_out matmul)
"""

import os
import sys

import numpy as np

for _p in ("/opt/trn_rl_repo",):
    if os.path.isdir(_p) and _p not in sys.path:
        sys.path.insert(0, _p)

import concourse.bass as bass  # noqa: E402
import concourse.bacc as bacc  # noqa: E402
import concourse.tile as tile  # noqa: E402
from concourse import mybir  # noqa: E402
from concourse import bass_utils  # noqa: E402

HEADS = 12
DH = 64
WINDOW = 65
PAD = WINDOW // 2  # 32
E = 768
L = 2048
N_CORES = 8
LLOC = L // N_CORES           # 256 queries per core
LEXT = LLOC + 2 * PAD         # 320 token rows per core (with halo)
KC = E // 128                 # 6 contraction chunks
NBLK = LLOC // 128            # (kept for test.py's emulation)
F32 = mybir.dt.float32
F32R = mybir.dt.float32r
BF16 = mybir.dt.bfloat16

# compute dtype for matmul operands: "f32r" (TF32-like, fp32 storage) or
# "bf16" (half storage -> half DMA bytes, FWL weight loads, 2x DVE modes)
CDT_MODE = os.environ.get("KERNEL_CDT", "bf16")

_CACHE = {}

# window chunks: (row offset, rows) — aligned with the three v tiles
WCHUNKS = ((0, 128), (128, 128), (256, 64))


def _build_module(reps=1, loop_iters=None):
    bf = CDT_MODE == "bf16"
    ddt = BF16 if bf else F32       # dram storage dtype for casted inputs
    cdt = BF16 if bf else F32R      # sbuf compute dtype for matmul operands
    nc = bacc.Bacc(
        "TRN2",
        target_bir_lowering=False,
        debug=False,
        num_devices=N_CORES,
    )
    xT = nc.dram_tensor("xT", [E, LEXT], ddt, kind="ExternalInput").ap()
    wqkv = nc.dram_tensor("w_qkv", [E, 3 * HEADS * DH], ddt, kind="ExternalInput").ap()
    wout = nc.dram_tensor("w_out", [HEADS * DH, E], ddt, kind="ExternalInput").ap()
    bout = nc.dram_tensor("b_out", [E], ddt, kind="ExternalInput").ap()
    mask_d = [
        nc.dram_tensor(f"mask{c}", [p, LLOC], ddt, kind="ExternalInput").ap()
        for c, (_, p) in enumerate(WCHUNKS)
    ]
    onesc_d = nc.dram_tensor("ones_col", [128, 1], ddt, kind="ExternalInput").ap()
    onesr_d = nc.dram_tensor("ones_row", [1, 128], ddt, kind="ExternalInput").ap()
    out_d = nc.dram_tensor("out", [LLOC, E], F32, kind="ExternalOutput").ap()

    with tile.TileContext(nc) as tc:
        with (
            tc.tile_pool(name="wq", bufs=KC) as wq_pool,
            tc.tile_pool(name="wo", bufs=KC) as wo_pool,
            tc.tile_pool(name="xt", bufs=KC) as xt_pool,
            tc.tile_pool(name="const", bufs=1) as cpool,
            tc.tile_pool(name="qk", bufs=2 * HEADS) as qk_pool,
            tc.tile_pool(name="vsb", bufs=3) as v_pool,
            tc.tile_pool(name="ex", bufs=6) as ex_pool,
            tc.tile_pool(name="opair", bufs=KC) as op_pool,
            tc.tile_pool(name="rb", bufs=2) as rb_pool,
            tc.tile_pool(name="rden", bufs=2) as rden_pool,
            tc.tile_pool(name="fin", bufs=2) as fin_pool,
            tc.tile_pool(name="psA", bufs=2, space="PSUM") as psA,
            tc.tile_pool(name="psS", bufs=3, space="PSUM") as psS,
            tc.tile_pool(name="psD", bufs=1, space="PSUM") as psD,
            tc.tile_pool(name="psO", bufs=2, space="PSUM") as psO,
        ):
            # ---- constants ----
            def _cd(ap):
                return ap if bf else ap.bitcast(F32R)

            masks = []
            for c, (_, p) in enumerate(WCHUNKS):
                m = cpool.tile([p, LLOC], cdt, tag=f"mask{c}", name=f"mask{c}")
                nc.sync.dma_start(m[:], _cd(mask_d[c]))
                masks.append(m)
            ones_col = cpool.tile([128, 1], cdt, tag="onesc")
            nc.sync.dma_start(ones_col[:], _cd(onesc_d))
            ones_row = cpool.tile([1, 128], cdt, tag="onesr")
            nc.sync.dma_start(ones_row[:], _cd(onesr_d))
            b_sb = cpool.tile([1, E], cdt, tag="bout")
            nc.sync.dma_start(b_sb[0:1, :], _cd(bout[None, :]))

            if loop_iters is None:
                for rep in range(reps):
                    _emit_body(nc, tc, locals())
            else:
                rep = 0
                with tc.For_i(
                    0, loop_iters, 1,
                    hint_engines=(mybir.EngineType.PE,
                                  mybir.EngineType.DVE,
                                  mybir.EngineType.Activation),
                ) as _i:
                    _emit_body(nc, tc, locals())

    nc.compile()
    return nc


def _emit_body(nc, tc, env):
    (xT, wqkv, wout, out_d) = (env["xT"], env["wqkv"], env["wout"], env["out_d"])
    (wq_pool, wo_pool, xt_pool, qk_pool, v_pool, ex_pool, op_pool,
     rb_pool, rden_pool, fin_pool, psA, psS, psD, psO) = (
        env["wq_pool"], env["wo_pool"], env["xt_pool"], env["qk_pool"],
        env["v_pool"], env["ex_pool"], env["op_pool"], env["rb_pool"],
        env["rden_pool"], env["fin_pool"], env["psA"], env["psS"],
        env["psD"], env["psO"],
    )
    masks, ones_col, ones_row, b_sb = (
        env["masks"], env["ones_col"], env["ones_row"], env["b_sb"]
    )
    rep = env["rep"]
    cdt, _cd = env["cdt"], env["_cd"]
    EXPF = mybir.ActivationFunctionType.Exp

    # ---- input DMAs ----
    xt_sb = []
    for c in range(KC):
        t = xt_pool.tile([128, LEXT], cdt, tag="xt", name=f"xt{rep}_{c}")
        nc.sync.dma_start(t[:], _cd(xT[c * 128:(c + 1) * 128, :]))
        xt_sb.append(t)
    wq_sb = []
    for c in range(KC):
        t = wq_pool.tile([128, 3 * HEADS * DH], cdt, tag="wq", name=f"wq{rep}_{c}")
        nc.sync.dma_start(t[:], _cd(wqkv[c * 128:(c + 1) * 128, :]))
        wq_sb.append(t)
    wo_sb = []
    for c in range(KC):
        t = wo_pool.tile([128, E], cdt, tag="wo", name=f"wo{rep}_{c}")
        nc.sync.dma_start(t[:], _cd(wout[c * 128:(c + 1) * 128, :]))
        wo_sb.append(t)

    # ---- 1. q/k projection (transposed layout) ----
    qTs, kTs = [], []
    for h in range(HEADS):
        ps = psA.tile([128, LEXT], F32, tag="proj", name=f"psqk{rep}_{h}")
        for c in range(KC):
            nc.tensor.matmul(
                ps[:],
                wq_sb[c][:, h * 3 * DH:h * 3 * DH + 2 * DH],
                xt_sb[c][:],
                start=(c == 0),
                stop=(c == KC - 1),
            )
        qT = qk_pool.tile([64, LLOC], cdt, tag="qk", name=f"qT{rep}_{h}")
        nc.vector.tensor_copy(qT[:], ps[0:DH, PAD:PAD + LLOC])
        kT = qk_pool.tile([64, LEXT], cdt, tag="qk", name=f"kT{rep}_{h}")
        nc.vector.tensor_copy(kT[:], ps[DH:2 * DH, :])
        qTs.append(qT)
        kTs.append(kT)

    # ---- 2. v projection (natural layout), all heads batched ----
    v_sb = []
    for t_i in range(3):
        P = 128 if t_i < 2 else 64
        vt = v_pool.tile([P, HEADS * DH], cdt, tag="vsb", name=f"v{rep}_{t_i}")
        for g in range(2):  # two groups of 6 heads, N=384 each
            vps = psA.tile([128, 384], F32, tag="proj", name=f"psv{rep}_{t_i}_{g}")
            for c in range(KC):
                w3 = wq_sb[c][:].rearrange("p (h f) -> p h f", h=HEADS)
                nc.tensor.matmul(
                    vps[0:P, :],
                    xt_sb[c][:, t_i * 128:t_i * 128 + P],
                    w3[:, 6 * g:6 * (g + 1), 2 * DH:3 * DH],
                    start=(c == 0),
                    stop=(c == KC - 1),
                )
            nc.vector.tensor_copy(vt[:, g * 384:(g + 1) * 384], vps[0:P, :])
        v_sb.append(vt)

    # ---- 3. banded attention: one 256-query block per head ----
    op_tiles = [
        op_pool.tile([128, LLOC], cdt, tag="opair", name=f"opair{rep}_{i}")
        for i in range(KC)
    ]
    for h in range(HEADS):
        exs = []
        for c, (off, p) in enumerate(WCHUNKS):
            sps = psS.tile([128, LLOC], F32, tag="s", name=f"s{rep}_{h}_{c}")
            nc.tensor.matmul(
                sps[0:p, :],
                kTs[h][:, off:off + p],
                qTs[h][:],
                start=True,
                stop=True,
            )
            ex = ex_pool.tile([p, LLOC], cdt, tag="ex", name=f"ex{rep}_{h}_{c}")
            nc.scalar.activation(ex[:], sps[0:p, :], EXPF, scale=0.125)
            nc.vector.tensor_mul(ex[:], ex[:], masks[c][:])
            exs.append(ex)

        dps = psD.tile([1, LLOC], F32, tag="den", name=f"den{rep}_{h}")
        for c, (off, p) in enumerate(WCHUNKS):
            nc.tensor.matmul(
                dps[:],
                ones_col[0:p, 0:1],
                exs[c][:],
                start=(c == 0),
                stop=(c == 2),
            )
        rden = rden_pool.tile([1, LLOC], cdt, tag="rden", name=f"rden{rep}_{h}")
        with nc.allow_low_precision(reason="f32r view feeding PE broadcast"):
            nc.vector.reciprocal(rden[:], dps[:])
        rbps = psO.tile([64, LLOC], F32, tag="o", name=f"rbps{rep}_{h}")
        nc.tensor.matmul(
            rbps[:], ones_row[0:1, 0:64], rden[:], start=True, stop=True
        )
        rb = rb_pool.tile([64, LLOC], cdt, tag="rb", name=f"rb{rep}_{h}")
        nc.scalar.copy(rb[:], rbps[:])

        ops = psO.tile([64, LLOC], F32, tag="o", name=f"o{rep}_{h}")
        for c, (off, p) in enumerate(WCHUNKS):
            nc.tensor.matmul(
                ops[:],
                v_sb[c][:, h * DH:(h + 1) * DH],
                exs[c][:],
                start=(c == 0),
                stop=(c == 2),
            )
        opt = op_tiles[h // 2]
        nc.vector.tensor_mul(
            opt[(h % 2) * 64:(h % 2) * 64 + 64, :],
            ops[:],
            rb[:],
        )

    # ---- 4. output projection ----
    for t_i in range(2):
        fsb = fin_pool.tile([128, E], F32, tag="fin", name=f"fin{rep}_{t_i}")
        for n0, nw in ((0, 512), (512, 256)):
            fps = psA.tile([128, 512], F32, tag="proj", name=f"psf{rep}_{t_i}_{n0}")
            for c in range(KC):
                nc.tensor.matmul(
                    fps[:, 0:nw],
                    op_tiles[c][:, t_i * 128:(t_i + 1) * 128],
                    wo_sb[c][:, n0:n0 + nw],
                    start=(c == 0),
                    stop=False,
                )
            nc.tensor.matmul(
                fps[:, 0:nw],
                ones_row[0:1, 0:128],
                b_sb[0:1, n0:n0 + nw],
                start=False,
                stop=True,
            )
            nc.scalar.copy(fsb[:, n0:n0 + nw], fps[:, 0:nw])
        nc.sync.dma_start(out_d[t_i * 128:(t_i + 1) * 128, :], fsb[:])


def _get_module(reps=1, loop_iters=None):
    key = (reps, loop_iters)
    if key not in _CACHE:
        _CACHE[key] = _build_module(reps, loop_iters)
    return _CACHE[key]


def _make_masks():
    j = np.arange(LEXT)[:, None]
    i = np.arange(LLOC)[None, :]
    m = ((j >= i) & (j <= i + 2 * PAD)).astype(np.float32)
    return [np.ascontiguousarray(m[off:off + p]) for off, p in WCHUNKS]


def _prepare_in_maps(x, w_qkv, w_out, b_out):
    import ml_dtypes

    ndt = ml_dtypes.bfloat16 if CDT_MODE == "bf16" else np.float32
    x2 = np.asarray(x, dtype=np.float32).reshape(L, E)
    xp = np.zeros((L + 2 * PAD, E), dtype=np.float32)
    xp[PAD:PAD + L] = x2
    xp = xp.astype(ndt)
    masks = [m.astype(ndt) for m in _make_masks()]
    w_qkv = np.ascontiguousarray(np.asarray(w_qkv, dtype=np.float32).astype(ndt))
    w_out = np.ascontiguousarray(np.asarray(w_out, dtype=np.float32).astype(ndt))
    b_out = np.ascontiguousarray(np.asarray(b_out, dtype=np.float32).astype(ndt))
    in_maps = []
    for i in range(N_CORES):
        xTi = np.ascontiguousarray(xp[i * LLOC:i * LLOC + LEXT].T)
        m = {
            "xT": xTi,
            "w_qkv": w_qkv,
            "w_out": w_out,
            "b_out": b_out,
        }
        for c in range(3):
            m[f"mask{c}"] = masks[c]
        m["ones_col"] = np.ones((128, 1), ndt)
        m["ones_row"] = np.ones((1, 128), ndt)
        in_maps.append(m)
    return in_maps


def _run(in_maps, trace=False, trace_cores=None, reps=1, loop_iters=None):
    nc = _get_module(reps, loop_iters)
    return bass_utils.run_bass_kernel_spmd(
        nc,
        in_maps,
        core_ids=list(range(N_CORES)),
        trace=trace,
        trace_cores=trace_cores,
    )


def kernel(x, w_qkv, w_out, b_out):
    res = _run(_prepare_in_maps(x, w_qkv, w_out, b_out))
    y = np.concatenate([res.results[i]["out"] for i in range(N_CORES)], axis=0)
    return y.reshape(1, L, E).astype(np.float32)


def kernel_profiled(x, w_qkv, w_out, b_out, trace_cores=None):
    res = _run(
        _prepare_in_maps(x, w_qkv, w_out, b_out),
        trace=True,
        trace_cores=trace_cores,
    )
    y = np.concatenate([res.results[i]["out"] for i in range(N_CORES)], axis=0)
    return y.reshape(1, L, E).astype(np.float32), res
